# revision 26
# baseline (speedup 1.0000x reference)
"""Trainium2 Bass kernel for nn_AttentionMap (dense self-attention map over
feature maps): out = gamma * (v @ softmax(q^T k)^T) + x, with q/k/v 1x1-conv
projections of x.

Sharding: data-parallel over batch B=8 -> one batch element per NeuronCore.

Per-core algorithm (N = H*W = 2304, C = 256, CR = 32):
  - q = w1 @ x + b1, k = w2 @ x + b2            [32, N] bf16
  - scores are computed TRANSPOSED: sT[j, i] = sum_d k[d,j] q[d,i], so the
    softmax contraction axis j lands on PSUM partitions; exp() is applied on
    PSUM eviction with no max-subtraction (|s| < ~25, safe in fp32/bf16).
    The K=32 contraction is row-tiled 4x (k/q replicated to partitions
    32..128) so the chunk matmuls stream concurrently.
  - vT[j, 0:256] = gamma*(w3 @ x + b3)^T; vt_sb column 256 is a preset 1.0
    (softmax denominator via the ones-column trick).
  - refineT[i, :] = sum_j E[j, i] * vT[j, :] accumulated on PSUM; column 256
    is then Z_i = sum_j exp(s[i, j]).
  - outT[i, c] = refineT[i, c] * (1/Z_i) + xT[i, c] in one fused DVE op.
Host transposes outT back to [C, H, W].

Schedule (v2): one fused streaming loop over the 18 j-blocks.  Per block the
2304 score columns split into PSUM chunks A0/A1 (ACT: exact exp, 2 instrs)
and B0/BV (DVE: Schraudolph exp via int16 bit trick, 2 instrs), balanced so
both engines run ~1.5us/block.  vT shares the BV PSUM bank (cols 256:512).
Three refine i-tiles accumulate PSUM-resident (banks 5-7) at lag-1 through
the whole loop, keeping the PE dense enough to hold the HAM clock at 8/8.
Phase B streams the remaining 15 i-tiles as back-to-back LDW+MM pairs.
"""

import json

import numpy as np
import ml_dtypes

import concourse.bass as bass
import concourse.mybir as mybir
import concourse.tile as tile
from concourse import bass2jax as _b2j
from concourse.bass_utils import compile_bir_kernel as _orig_compile_bir_kernel
from concourse.bass_utils import run_bass_kernel_spmd

BF16 = ml_dtypes.bfloat16
F32 = mybir.dt.float32
BF = mybir.dt.bfloat16
I16 = mybir.dt.int16
AF = mybir.ActivationFunctionType
AL = mybir.AluOpType

B, C, H, W = 8, 256, 48, 48
N = H * W            # 2304
CR = C // 8          # 32
NT = N // 128        # 18 tiles of 128 along both i and j
KT = C // 128        # 2 k-tiles over channels
NH = N // 2          # 1152

# exp column split: ACT takes [0:S_ACT) (exact exp), DVE the rest
# (Schraudolph).  Chunk sizes are PSUM-bank shaped: A0/A1 = 512 each,
# B0 = 1024, BV = 256 (+256 cols of vT in the same bank).
S_ACT = 1024
N_ACC = 3            # PSUM-resident refine accumulators (phase A)

WB_W = 2 * CR + C    # packed weight columns: w1t | w2t | (g*w3)t

# Schraudolph: bf16 bit pattern of e^s is int16(s*128/ln2 + 16250.5)
SCH_MUL = 184.66502
SCH_ADD = 16250.5

# ---------------------------------------------------------------------------
# Workaround for this walrus build's per-instruction sync-wait limit (it
# rejects any instruction carrying more than one sem wait with "Too many
# sync wait commands").  Tile's scheduler freely emits multi-wait
# instructions, so rewrite the BIR JSON just before the walrus compile:
# hoist all but the last wait of each instruction onto same-engine NoOps
# inserted directly before it.


def _split_multiwait_bir(bir_json: bytes) -> bytes:
    m = json.loads(bir_json)
    n = 0
    for fn in m["functions"]:
        for blk in fn["blocks"]:
            out = []
            for ins in blk["instructions"]:
                si = ins.get("sync_info")
                waits = (si or {}).get("on_wait") or []
                if len(waits) > 1:
                    for w in waits[:-1]:
                        n += 1
                        out.append({
                            "debug": ins.get("debug", 0),
                            "engine": ins["engine"],
                            "ins": [],
                            "outs": [],
                            "name": f"{ins['name']}_sw{n}",
                            "opcode": "NoOp",
                            "sync_info": {"on_wait": [w], "on_update": []},
                        })
                    si["on_wait"] = [waits[-1]]
                out.append(ins)
            blk["instructions"] = out
    return json.dumps(m).encode()


def _patched_compile_bir_kernel(bir_json, tmpdir, neff_name="file.neff"):
    return _orig_compile_bir_kernel(_split_multiwait_bir(bytes(bir_json)),
                                    tmpdir, neff_name)


_b2j.compile_bir_kernel = _patched_compile_bir_kernel
# ---------------------------------------------------------------------------


def _build_program():
    nc = bass.Bass("TRN2", target_bir_lowering=False, debug=False)

    def din(name, shape, dt):
        return nc.dram_tensor(name, shape, dt, kind="ExternalInput").ap()

    wb_d = din("wb", [128, KT, WB_W], BF)    # w1^T | w2^T | (g*w3)^T
    bsc_d = din("bsc", [CR, 2], F32)         # b1 | b2
    x_d = din("x", [128, KT, N], BF)         # x[c, n]: c = kt*128 + p
    xt_d = din("xt", [128, NT, C], F32)      # x^T[i, c] + g*b3[c] (b3 folded)
    ot_d = nc.dram_tensor("ot", [128, NT, C], F32, kind="ExternalOutput").ap()

    with tile.TileContext(nc) as tc:
        with tc.tile_pool(name="const", bufs=1) as cp:
            # DMA ring order: tiny weights first, then the two x halves, each
            # a single InstDMACopy fanned across all 16 SDMA engines so x-h0
            # lands at full bandwidth (xt is held back until x drains)
            wb_sb = cp.tile([128, KT, WB_W], BF)
            nc.sync.dma_start(wb_sb[:], wb_d[:])
            bsc_sb = cp.tile([CR, 2], F32)
            nc.sync.dma_start(bsc_sb[:], bsc_d[:])
            x_sb = cp.tile([128, KT, N], BF)
            for h in range(2):
                nc.sync.dma_start(x_sb[:, :, h * NH:(h + 1) * NH],
                                  x_d[:, :, h * NH:(h + 1) * NH])
            xt_sb = cp.tile([128, NT, C], F32)
            zb_sb = cp.tile([128, 1], F32)
            nc.vector.memset(zb_sb[:], 0.0)
            ws_sb = cp.tile([128, 512], BF)   # zero scratch: PE warm-up feed
            nc.vector.memset(ws_sb[:], 0.0)
            # dummy exp: pull the ACT exp table load into the DMA wait
            zs_sb = cp.tile([128, 1], F32)
            nc.scalar.activation(zs_sb[:], zb_sb[:], AF.Exp, bias=zb_sb[:])

            w1t = wb_sb[:, :, 0:CR]
            w2t = wb_sb[:, :, CR:2 * CR]
            w3e = wb_sb[:, :, 2 * CR:WB_W]

            # rows 0:32 hold q/k; rows 32:128 are zeroed so score matmuls can
            # run full-K=128 (HAM's activity monitor under-counts K=32
            # row-group matmuls and keeps the PE clock-gated at 1.2 GHz;
            # zero-padded full-K streams at the same cols/cycle but counts).
            q_sb = cp.tile([128, N], BF)
            k_sb = cp.tile([128, N], BF)
            # zero-fill on the otherwise-idle GPSIMD so the DVE is free for
            # the q/k evictions the moment x lands
            nc.gpsimd.memset(k_sb[:], 0.0)
            nc.gpsimd.memset(q_sb[:], 0.0)
            # pad the GPSIMD queue so the xt-delay WAR (below) releases only
            # after both x halves have drained their HBM bandwidth
            xpad_sb = cp.tile([128, N], BF)
            nc.gpsimd.memset(xpad_sb[:], 0.0)
            vt_sb = cp.tile([128, NT, C + 1], BF)
            nc.vector.memset(vt_sb[:, :, C:C + 1], 1.0)  # ones col: Z trick
            e_sb = cp.tile([128, NT, N], BF)
            xwar_sb = cp.tile([1, 2], F32)

            # ---- prologue: q, k (+ row replicas via SBUF-SBUF DMA) ------
            with tc.tile_pool(name="pqk", bufs=2, space="PSUM") as pqk:
                # dummy matmuls on a memset scratch (no DMA dependency) to
                # lift the PE HAM clock-gate to 8/8 while x is in flight;
                # own tag so the q/k tiles never wait on the warm-up slot
                warm = pqk.tile([128, 512], F32, tag="warm", bufs=1,
                                name="warm")
                # enough to bridge the DMA wait: HAM un-throttles after
                # ~3.4us of sustained activity, so the q/k matmuls (and the
                # first phase-A periods) run at 2.4 GHz instead of 1.2
                for i in range(12):
                    nc.tensor.matmul(
                        warm[:], ws_sb[:, 0:128], ws_sb[:],
                        start=True, stop=True,
                    )
                for h in range(2):
                    qp = pqk.tile([CR, NH], F32, tag="pqk", name=f"qp{h}")
                    kp = pqk.tile([CR, NH], F32, tag="pqk", name=f"kp{h}")
                    # k first: its (slower, DVE) eviction starts earliest;
                    # evictions are split in two column chunks so the first
                    # chunk drains while the PE finishes the rest
                    for ps, wt in ((kp, w2t), (qp, w1t)):
                        for c0, cw in ((0, 512), (512, 512), (1024, 128)):
                            for kt in range(KT):
                                nc.tensor.matmul(
                                    ps[:, c0:c0 + cw],
                                    wt[:, kt, :],
                                    x_sb[:, kt, h * NH + c0:h * NH + c0 + cw],
                                    start=(kt == 0), stop=(kt == KT - 1),
                                )
                    for c0, c1 in ((0, 512), (512, NH)):
                        nc.vector.tensor_scalar(
                            k_sb[0:CR, h * NH + c0:h * NH + c1],
                            kp[:, c0:c1], bsc_sb[:, 1:2], None, AL.add)
                        nc.scalar.activation(
                            q_sb[0:CR, h * NH + c0:h * NH + c1],
                            qp[:, c0:c1], AF.Identity, bias=bsc_sb[:, 0:1])


            # ---- fused phase A ------------------------------------------
            with tc.tile_pool(name="pacc", bufs=1, space="PSUM") as pacc:
                accs = [pacc.tile([128, C + 1], F32, tag=f"acc{i}",
                                  name=f"acc{i}") for i in range(N_ACC)]

                with tc.tile_pool(name="pa", bufs=1, space="PSUM") as pa, \
                     tc.tile_pool(name="pb", bufs=1, space="PSUM") as pb:
                    for jt in range(NT):
                        js = slice(jt * 128, (jt + 1) * 128)
                        a0 = pa.tile([128, 1024], F32, tag="a0", name=f"a0_{jt}")
                        b0 = pb.tile([128, 1024], F32, tag="b0", name=f"b0_{jt}")
                        bv = pb.tile([128, 512], F32, tag="bv", name=f"bv_{jt}")

                        def smm(dst, c0, cw):
                            nc.tensor.matmul(
                                dst,
                                k_sb[:, js],
                                q_sb[:, c0:c0 + cw],
                                start=True, stop=True,
                            )

                        # PE emission order matters: each engine queue is
                        # FIFO, so instructions whose deps clear earliest go
                        # first (bv's score chunk waits on the previous
                        # iteration's vt eviction and goes last).
                        smm(a0[:, 0:512], 0, 512)
                        smm(a0[:, 512:1024], 512, 512)
                        smm(b0[:, 0:512], 1024, 512)
                        smm(b0[:, 512:1024], 1536, 512)

                        # vT for this j-block, colocated in bv's bank
                        # (gamma*b3 is folded into xt on the host)
                        nc.tensor.matmul(bv[:, 256:512], x_sb[:, 0, js],
                                         w3e[:, 0, :], start=True, stop=False)
                        nc.tensor.matmul(bv[:, 256:512], x_sb[:, 1, js],
                                         w3e[:, 1, :], start=False, stop=True)

                        # PSUM-resident refine accumulators, lag-1
                        if jt >= 1:
                            for it in range(N_ACC):
                                nc.tensor.matmul(
                                    accs[it],
                                    e_sb[:, jt - 1, it * 128:(it + 1) * 128],
                                    vt_sb[:, jt - 1, :],
                                    start=(jt == 1), stop=False,
                                )
                        smm(bv[:, 0:256], 2048, 256)

                        nc.scalar.activation(e_sb[:, jt, 0:1024], a0[:],
                                             AF.Exp, bias=zb_sb[:])
                        nc.scalar.copy(vt_sb[:, jt, 0:C], bv[:, 256:512])
                        nc.vector.tensor_scalar(
                            e_sb[:, jt, 1024:2048].bitcast(I16), b0[:],
                            SCH_MUL, SCH_ADD, AL.mult, AL.add)
                        nc.vector.tensor_scalar(
                            e_sb[:, jt, 2048:2304].bitcast(I16), bv[:, 0:256],
                            SCH_MUL, SCH_ADD, AL.mult, AL.add)

                        if jt == 1:
                            # delay the (large, late-needed) xt fetch until x
                            # has the HBM bandwidth to itself: WAR on a dummy
                            # GPSIMD read holds the DMA back to ~mid-prologue
                            nc.gpsimd.tensor_copy(xwar_sb[:], xt_sb[0:1, 0, 0:2])
                            for p in range(4):
                                nc.sync.dma_start(
                                    xt_sb[:, p * 5:min(NT, 5 * p + 5), :],
                                    xt_d[:, p * 5:min(NT, 5 * p + 5), :])
                # ---- phase B (pa/pb banks freed; pacc still open so the
                # resident accumulators can finalize while phase B streams)
                with tc.tile_pool(name="prb", bufs=5, space="PSUM") as prb, \
                     tc.tile_pool(name="zo2", bufs=3) as zo2:

                    def fin(src, it):
                        zinv = zo2.tile([128, 1], F32, tag="zinv2",
                                        name=f"zi{it}")
                        nc.vector.reciprocal(zinv[:], src[:, C:C + 1])
                        o_sb = zo2.tile([128, C], F32, tag="osb2",
                                        name=f"ob{it}")
                        nc.vector.scalar_tensor_tensor(
                            o_sb[:], src[:, 0:C], zinv[:], xt_sb[:, it, :],
                            op0=AL.mult, op1=AL.add)
                        nc.sync.dma_start(ot_d[:, it, :], o_sb[:])

                    def btile(it):
                        r_ps = prb.tile([128, C + 1], F32, tag="prb",
                                        name=f"r{it}")
                        for jt in range(NT):
                            nc.tensor.matmul(
                                r_ps[:],
                                e_sb[:, jt, it * 128:(it + 1) * 128],
                                vt_sb[:, jt, :],
                                start=(jt == 0), stop=(jt == NT - 1),
                            )
                        fin(r_ps, it)

                    # first streamed tile goes ahead of the resident tails
                    # so the PE never idles at the phase boundary
                    btile(N_ACC)
                    for it in range(N_ACC):
                        nc.tensor.matmul(
                            accs[it],
                            e_sb[:, NT - 1, it * 128:(it + 1) * 128],
                            vt_sb[:, NT - 1, :],
                            start=False, stop=True,
                        )
                    for it in range(N_ACC):
                        fin(accs[it], it)
                    for it in range(N_ACC + 1, NT):
                        btile(it)

    return nc


_NC = None


def _get_nc():
    global _NC
    if _NC is None:
        _NC = _build_program()
    return _NC


def _prep_inputs(feat_map, w1, b1, w2, b2, w3, b3, gamma):
    g = float(np.asarray(gamma))
    wb = np.zeros((C, WB_W), np.float32)
    wb[:, 0:CR] = np.asarray(w1, np.float32).T
    wb[:, CR:2 * CR] = np.asarray(w2, np.float32).T
    wb[:, 2 * CR:WB_W] = g * np.asarray(w3, np.float32).T
    gb3 = g * np.asarray(b3, np.float32)
    shared = {
        "wb": np.ascontiguousarray(
            wb.reshape(KT, 128, WB_W).transpose(1, 0, 2)
        ).astype(BF16),
        "bsc": np.stack(
            [np.asarray(b1, np.float32), np.asarray(b2, np.float32)], axis=1
        ),
    }

    fm = np.asarray(feat_map, np.float32)
    in_maps = []
    for b in range(B):
        x = fm[b].reshape(C, N)
        m = dict(shared)
        m["x"] = np.ascontiguousarray(
            x.reshape(KT, 128, N).transpose(1, 0, 2)
        ).astype(BF16)
        m["xt"] = np.ascontiguousarray(
            (x.T + gb3[None, :]).reshape(NT, 128, C).transpose(1, 0, 2)
        )
        in_maps.append(m)
    return in_maps


def _run(inputs, trace=False):
    nc = _get_nc()
    in_maps = _prep_inputs(**inputs)
    res = run_bass_kernel_spmd(nc, in_maps, core_ids=list(range(B)), trace=trace)
    out = np.empty((B, C, H, W), np.float32)
    for b in range(B):
        ot = res.results[b]["ot"]                      # [128, NT, C]
        o_t = ot.transpose(1, 0, 2).reshape(N, C)      # outT[i, c]
        out[b] = o_t.T.reshape(C, H, W)
    return out, res


def kernel(**inputs) -> np.ndarray:
    out, _ = _run(inputs, trace=False)
    return out


# revision 27
# speedup vs baseline: 1.0058x; 1.0058x over previous
"""Trainium2 Bass kernel for nn_AttentionMap (dense self-attention map over
feature maps): out = gamma * (v @ softmax(q^T k)^T) + x, with q/k/v 1x1-conv
projections of x.

Sharding: data-parallel over batch B=8 -> one batch element per NeuronCore.

Per-core algorithm (N = H*W = 2304, C = 256, CR = 32):
  - q = w1 @ x + b1, k = w2 @ x + b2            [32, N] bf16
  - scores are computed TRANSPOSED: sT[j, i] = sum_d k[d,j] q[d,i], so the
    softmax contraction axis j lands on PSUM partitions; exp() is applied on
    PSUM eviction with no max-subtraction (|s| < ~25, safe in fp32/bf16).
    The K=32 contraction is row-tiled 4x (k/q replicated to partitions
    32..128) so the chunk matmuls stream concurrently.
  - vT[j, 0:256] = gamma*(w3 @ x + b3)^T; vt_sb column 256 is a preset 1.0
    (softmax denominator via the ones-column trick).
  - refineT[i, :] = sum_j E[j, i] * vT[j, :] accumulated on PSUM; column 256
    is then Z_i = sum_j exp(s[i, j]).
  - outT[i, c] = refineT[i, c] * (1/Z_i) + xT[i, c] in one fused DVE op.
Host transposes outT back to [C, H, W].

Schedule (v2): one fused streaming loop over the 18 j-blocks.  Per block the
2304 score columns split into PSUM chunks A0/A1 (ACT: exact exp, 2 instrs)
and B0/BV (DVE: Schraudolph exp via int16 bit trick, 2 instrs), balanced so
both engines run ~1.5us/block.  vT shares the BV PSUM bank (cols 256:512).
Three refine i-tiles accumulate PSUM-resident (banks 5-7) at lag-1 through
the whole loop, keeping the PE dense enough to hold the HAM clock at 8/8.
Phase B streams the remaining 15 i-tiles as back-to-back LDW+MM pairs.
"""

import json

import numpy as np
import ml_dtypes

import concourse.bass as bass
import concourse.mybir as mybir
import concourse.tile as tile
from concourse import bass2jax as _b2j
from concourse.bass_utils import compile_bir_kernel as _orig_compile_bir_kernel
from concourse.bass_utils import run_bass_kernel_spmd

BF16 = ml_dtypes.bfloat16
F32 = mybir.dt.float32
BF = mybir.dt.bfloat16
I16 = mybir.dt.int16
AF = mybir.ActivationFunctionType
AL = mybir.AluOpType

B, C, H, W = 8, 256, 48, 48
N = H * W            # 2304
CR = C // 8          # 32
NT = N // 128        # 18 tiles of 128 along both i and j
KT = C // 128        # 2 k-tiles over channels
NH = N // 2          # 1152

# exp column split: ACT takes [0:S_ACT) (exact exp), DVE the rest
# (Schraudolph).  Chunk sizes are PSUM-bank shaped: A0/A1 = 512 each,
# B0 = 1024, BV = 256 (+256 cols of vT in the same bank).
S_ACT = 1024
N_ACC = 3            # PSUM-resident refine accumulators (phase A)

WB_W = 2 * CR + C    # packed weight columns: w1t | w2t | (g*w3)t

# Schraudolph: bf16 bit pattern of e^s is int16(s*128/ln2 + 16250.5)
SCH_MUL = 184.66502
SCH_ADD = 16250.5

# ---------------------------------------------------------------------------
# Workaround for this walrus build's per-instruction sync-wait limit (it
# rejects any instruction carrying more than one sem wait with "Too many
# sync wait commands").  Tile's scheduler freely emits multi-wait
# instructions, so rewrite the BIR JSON just before the walrus compile:
# hoist all but the last wait of each instruction onto same-engine NoOps
# inserted directly before it.


def _split_multiwait_bir(bir_json: bytes) -> bytes:
    m = json.loads(bir_json)
    n = 0
    for fn in m["functions"]:
        for blk in fn["blocks"]:
            out = []
            for ins in blk["instructions"]:
                si = ins.get("sync_info")
                waits = (si or {}).get("on_wait") or []
                if len(waits) > 1:
                    for w in waits[:-1]:
                        n += 1
                        out.append({
                            "debug": ins.get("debug", 0),
                            "engine": ins["engine"],
                            "ins": [],
                            "outs": [],
                            "name": f"{ins['name']}_sw{n}",
                            "opcode": "NoOp",
                            "sync_info": {"on_wait": [w], "on_update": []},
                        })
                    si["on_wait"] = [waits[-1]]
                out.append(ins)
            blk["instructions"] = out
    return json.dumps(m).encode()


def _patched_compile_bir_kernel(bir_json, tmpdir, neff_name="file.neff"):
    return _orig_compile_bir_kernel(_split_multiwait_bir(bytes(bir_json)),
                                    tmpdir, neff_name)


_b2j.compile_bir_kernel = _patched_compile_bir_kernel
# ---------------------------------------------------------------------------


def _build_program():
    nc = bass.Bass("TRN2", target_bir_lowering=False, debug=False)

    def din(name, shape, dt):
        return nc.dram_tensor(name, shape, dt, kind="ExternalInput").ap()

    wb_d = din("wb", [128, KT, WB_W], BF)    # w1^T | w2^T | (g*w3)^T
    bsc_d = din("bsc", [CR, 2], F32)         # b1 | b2
    x_d = din("x", [128, KT, N], BF)         # x[c, n]: c = kt*128 + p
    xt_d = din("xt", [128, NT, C], F32)      # x^T[i, c] + g*b3[c] (b3 folded)
    ot_d = nc.dram_tensor("ot", [128, NT, C], F32, kind="ExternalOutput").ap()

    with tile.TileContext(nc) as tc:
        with tc.tile_pool(name="const", bufs=1) as cp:
            # DMA ring order: tiny weights first, then the two x halves, each
            # a single InstDMACopy fanned across all 16 SDMA engines so x-h0
            # lands at full bandwidth (xt is held back until x drains)
            wb_sb = cp.tile([128, KT, WB_W], BF)
            nc.sync.dma_start(wb_sb[:], wb_d[:])
            bsc_sb = cp.tile([CR, 2], F32)
            nc.sync.dma_start(bsc_sb[:], bsc_d[:])
            x_sb = cp.tile([128, KT, N], BF)
            for h in range(2):
                nc.sync.dma_start(x_sb[:, :, h * NH:(h + 1) * NH],
                                  x_d[:, :, h * NH:(h + 1) * NH])
            xt_sb = cp.tile([128, NT, C], F32)
            zb_sb = cp.tile([128, 1], F32)
            nc.vector.memset(zb_sb[:], 0.0)
            ws_sb = cp.tile([128, 512], BF)   # zero scratch: PE warm-up feed
            nc.vector.memset(ws_sb[:], 0.0)
            # dummy exp: pull the ACT exp table load into the DMA wait
            zs_sb = cp.tile([128, 1], F32)
            nc.scalar.activation(zs_sb[:], zb_sb[:], AF.Exp, bias=zb_sb[:])

            w1t = wb_sb[:, :, 0:CR]
            w2t = wb_sb[:, :, CR:2 * CR]
            w3e = wb_sb[:, :, 2 * CR:WB_W]

            # rows 0:32 hold q/k; rows 32:128 are zeroed so score matmuls can
            # run full-K=128 (HAM's activity monitor under-counts K=32
            # row-group matmuls and keeps the PE clock-gated at 1.2 GHz;
            # zero-padded full-K streams at the same cols/cycle but counts).
            q_sb = cp.tile([128, N], BF)
            k_sb = cp.tile([128, N], BF)
            # zero-fill on the otherwise-idle GPSIMD so the DVE is free for
            # the q/k evictions the moment x lands
            nc.gpsimd.memset(k_sb[:], 0.0)
            nc.gpsimd.memset(q_sb[:], 0.0)
            # pad the GPSIMD queue so the xt-delay WAR (below) releases only
            # after both x halves have drained their HBM bandwidth
            xpad_sb = cp.tile([128, N], BF)
            nc.gpsimd.memset(xpad_sb[:], 0.0)
            vt_sb = cp.tile([128, NT, C + 1], BF)
            nc.vector.memset(vt_sb[:, :, C:C + 1], 1.0)  # ones col: Z trick
            e_sb = cp.tile([128, NT, N], BF)
            xwar_sb = cp.tile([1, 2], F32)

            # ---- prologue: q, k (+ row replicas via SBUF-SBUF DMA) ------
            with tc.tile_pool(name="pqk", bufs=2, space="PSUM") as pqk:
                # dummy matmuls on a memset scratch (no DMA dependency) to
                # lift the PE HAM clock-gate to 8/8 while x is in flight;
                # own tag so the q/k tiles never wait on the warm-up slot
                warm = pqk.tile([128, 512], F32, tag="warm", bufs=1,
                                name="warm")
                # enough to bridge the DMA wait: HAM un-throttles after
                # ~3.4us of sustained activity, so the q/k matmuls (and the
                # first phase-A periods) run at 2.4 GHz instead of 1.2
                for i in range(12):
                    nc.tensor.matmul(
                        warm[:], ws_sb[:, 0:128], ws_sb[:],
                        start=True, stop=True,
                    )
                for h in range(2):
                    hs = slice(h * NH, (h + 1) * NH)
                    qp = pqk.tile([CR, NH], F32, tag="pqk", name=f"qp{h}")
                    kp = pqk.tile([CR, NH], F32, tag="pqk", name=f"kp{h}")
                    for ps, wt in ((qp, w1t), (kp, w2t)):
                        for c0, cw in ((0, 512), (512, 512), (1024, 128)):
                            for kt in range(KT):
                                nc.tensor.matmul(
                                    ps[:, c0:c0 + cw],
                                    wt[:, kt, :],
                                    x_sb[:, kt, h * NH + c0:h * NH + c0 + cw],
                                    start=(kt == 0), stop=(kt == KT - 1),
                                )
                    nc.scalar.activation(q_sb[0:CR, hs], qp[:], AF.Identity,
                                         bias=bsc_sb[:, 0:1])
                    nc.vector.tensor_scalar(k_sb[0:CR, hs], kp[:],
                                            bsc_sb[:, 1:2], None, AL.add)


            # ---- fused phase A ------------------------------------------
            with tc.tile_pool(name="pacc", bufs=1, space="PSUM") as pacc:
                accs = [pacc.tile([128, C + 1], F32, tag=f"acc{i}",
                                  name=f"acc{i}") for i in range(N_ACC)]

                with tc.tile_pool(name="pa", bufs=1, space="PSUM") as pa, \
                     tc.tile_pool(name="pb", bufs=1, space="PSUM") as pb:
                    for jt in range(NT):
                        js = slice(jt * 128, (jt + 1) * 128)
                        a0 = pa.tile([128, 1024], F32, tag="a0", name=f"a0_{jt}")
                        b0 = pb.tile([128, 1024], F32, tag="b0", name=f"b0_{jt}")
                        bv = pb.tile([128, 512], F32, tag="bv", name=f"bv_{jt}")

                        def smm(dst, c0, cw):
                            nc.tensor.matmul(
                                dst,
                                k_sb[:, js],
                                q_sb[:, c0:c0 + cw],
                                start=True, stop=True,
                            )

                        # PE emission order matters: each engine queue is
                        # FIFO, so instructions whose deps clear earliest go
                        # first (bv's score chunk waits on the previous
                        # iteration's vt eviction and goes last).
                        smm(a0[:, 0:512], 0, 512)
                        smm(a0[:, 512:1024], 512, 512)
                        smm(b0[:, 0:512], 1024, 512)
                        smm(b0[:, 512:1024], 1536, 512)

                        # vT for this j-block, colocated in bv's bank
                        # (gamma*b3 is folded into xt on the host)
                        nc.tensor.matmul(bv[:, 256:512], x_sb[:, 0, js],
                                         w3e[:, 0, :], start=True, stop=False)
                        nc.tensor.matmul(bv[:, 256:512], x_sb[:, 1, js],
                                         w3e[:, 1, :], start=False, stop=True)

                        # PSUM-resident refine accumulators, lag-1
                        if jt >= 1:
                            for it in range(N_ACC):
                                nc.tensor.matmul(
                                    accs[it],
                                    e_sb[:, jt - 1, it * 128:(it + 1) * 128],
                                    vt_sb[:, jt - 1, :],
                                    start=(jt == 1), stop=False,
                                )
                        smm(bv[:, 0:256], 2048, 256)

                        nc.scalar.activation(e_sb[:, jt, 0:1024], a0[:],
                                             AF.Exp, bias=zb_sb[:])
                        nc.scalar.copy(vt_sb[:, jt, 0:C], bv[:, 256:512])
                        nc.vector.tensor_scalar(
                            e_sb[:, jt, 1024:2048].bitcast(I16), b0[:],
                            SCH_MUL, SCH_ADD, AL.mult, AL.add)
                        nc.vector.tensor_scalar(
                            e_sb[:, jt, 2048:2304].bitcast(I16), bv[:, 0:256],
                            SCH_MUL, SCH_ADD, AL.mult, AL.add)

                        if jt == 1:
                            # delay the (large, late-needed) xt fetch until x
                            # has the HBM bandwidth to itself: WAR on a dummy
                            # GPSIMD read holds the DMA back to ~mid-prologue
                            nc.gpsimd.tensor_copy(xwar_sb[:], xt_sb[0:1, 0, 0:2])
                            for p in range(4):
                                nc.sync.dma_start(
                                    xt_sb[:, p * 5:min(NT, 5 * p + 5), :],
                                    xt_d[:, p * 5:min(NT, 5 * p + 5), :])
                # ---- phase B (pa/pb banks freed; pacc still open so the
                # resident accumulators can finalize while phase B streams)
                with tc.tile_pool(name="prb", bufs=5, space="PSUM") as prb, \
                     tc.tile_pool(name="zo2", bufs=3) as zo2:

                    def fin(src, it):
                        zinv = zo2.tile([128, 1], F32, tag="zinv2",
                                        name=f"zi{it}")
                        nc.vector.reciprocal(zinv[:], src[:, C:C + 1])
                        o_sb = zo2.tile([128, C], F32, tag="osb2",
                                        name=f"ob{it}")
                        nc.vector.scalar_tensor_tensor(
                            o_sb[:], src[:, 0:C], zinv[:], xt_sb[:, it, :],
                            op0=AL.mult, op1=AL.add)
                        nc.sync.dma_start(ot_d[:, it, :], o_sb[:])

                    def btile(it):
                        r_ps = prb.tile([128, C + 1], F32, tag="prb",
                                        name=f"r{it}")
                        for jt in range(NT):
                            nc.tensor.matmul(
                                r_ps[:],
                                e_sb[:, jt, it * 128:(it + 1) * 128],
                                vt_sb[:, jt, :],
                                start=(jt == 0), stop=(jt == NT - 1),
                            )
                        fin(r_ps, it)

                    # first streamed tile goes ahead of the resident tails
                    # so the PE never idles at the phase boundary
                    btile(N_ACC)
                    for it in range(N_ACC):
                        nc.tensor.matmul(
                            accs[it],
                            e_sb[:, NT - 1, it * 128:(it + 1) * 128],
                            vt_sb[:, NT - 1, :],
                            start=False, stop=True,
                        )
                    for it in range(N_ACC):
                        fin(accs[it], it)
                    for it in range(N_ACC + 1, NT):
                        btile(it)

    return nc


_NC = None


def _get_nc():
    global _NC
    if _NC is None:
        _NC = _build_program()
    return _NC


def _prep_inputs(feat_map, w1, b1, w2, b2, w3, b3, gamma):
    g = float(np.asarray(gamma))
    wb = np.zeros((C, WB_W), np.float32)
    wb[:, 0:CR] = np.asarray(w1, np.float32).T
    wb[:, CR:2 * CR] = np.asarray(w2, np.float32).T
    wb[:, 2 * CR:WB_W] = g * np.asarray(w3, np.float32).T
    gb3 = g * np.asarray(b3, np.float32)
    shared = {
        "wb": np.ascontiguousarray(
            wb.reshape(KT, 128, WB_W).transpose(1, 0, 2)
        ).astype(BF16),
        "bsc": np.stack(
            [np.asarray(b1, np.float32), np.asarray(b2, np.float32)], axis=1
        ),
    }

    fm = np.asarray(feat_map, np.float32)
    in_maps = []
    for b in range(B):
        x = fm[b].reshape(C, N)
        m = dict(shared)
        m["x"] = np.ascontiguousarray(
            x.reshape(KT, 128, N).transpose(1, 0, 2)
        ).astype(BF16)
        m["xt"] = np.ascontiguousarray(
            (x.T + gb3[None, :]).reshape(NT, 128, C).transpose(1, 0, 2)
        )
        in_maps.append(m)
    return in_maps


def _run(inputs, trace=False):
    nc = _get_nc()
    in_maps = _prep_inputs(**inputs)
    res = run_bass_kernel_spmd(nc, in_maps, core_ids=list(range(B)), trace=trace)
    out = np.empty((B, C, H, W), np.float32)
    for b in range(B):
        ot = res.results[b]["ot"]                      # [128, NT, C]
        o_t = ot.transpose(1, 0, 2).reshape(N, C)      # outT[i, c]
        out[b] = o_t.T.reshape(C, H, W)
    return out, res


def kernel(**inputs) -> np.ndarray:
    out, _ = _run(inputs, trace=False)
    return out
